# revision 10
# baseline (speedup 1.0000x reference)
"""BiLSTM-CRF NLL kernel for 8 trn2 NeuronCores.

Data-parallel over batch: 8 cores x 16 batch elements. Each core runs the
whole model on-device:
  1. xg = W_ih @ x^T + b for both directions (big GEMM, staged to DRAM f16)
  2. BiLSTM scan, fwd+bwd interleaved per step; gates packed (128, 8x16)
     with the gate dimension on partitions, h history kept in SBUF (f16)
  3. emissions^T = W_e @ lstm_out^T (+b_e), exp() for the CRF
  4. CRF forward scan in exp space: E_{t+1} = em_t * (expT.T @ E_t) with
     periodic renormalization; masked steps freeze E via copy_predicated
Host does: embedding gather (f16), weight reordering, the gold-path score
(numerator) from the returned emissions, and the final reduction.
"""
import numpy as np

T, B = 512, 128
VOCAB, EMB, HID, NCLS = 32000, 256, 512, 25
H = HID // 2
PAD = 1
NCORES = 8
BS = B // NCORES          # 16
TMIN = T // 2             # lengths >= 256 -> mask true for t < 256
RENORM = 8                # renormalize E every RENORM steps
BLK = 32                  # timesteps per xg block / emissions N-chunk

_CACHE = {}


def _gate_perm():
    # pytorch gate order [i, f, g, o] -> device order [i, f, o, g]
    return np.concatenate([np.arange(0, 2 * H), np.arange(3 * H, 4 * H),
                           np.arange(2 * H, 3 * H)])


def _build_bass(t_dev=T, tmin=TMIN):
    from contextlib import ExitStack
    import concourse.bacc as bacc
    import concourse.tile as tile
    from concourse import mybir

    F32 = mybir.dt.float32
    F16 = mybir.dt.float16
    AF = mybir.ActivationFunctionType
    nblk = t_dev // BLK
    ntok = t_dev * BS
    nmask = t_dev - tmin

    nc = bacc.Bacc(None, target_bir_lowering=False)
    dt = nc.dram_tensor
    x16 = dt("x16", [2, ntok, 128], F16, kind="ExternalInput")
    wihT = [dt(f"wihT{d}", [128, 2 * 4 * H], F16, kind="ExternalInput")
            for d in range(2)]
    whhT = [dt(f"whhT{d}", [128, 2 * 4 * H], F16, kind="ExternalInput")
            for d in range(2)]
    bias = [dt(f"bias{d}", [128, 8], F32, kind="ExternalInput")
            for d in range(2)]
    weT = dt("weT", [128, 4 * NCLS], F16, kind="ExternalInput")
    b_e_t = dt("b_e", [NCLS, 1], F32, kind="ExternalInput")
    exp_start_t = dt("exp_start", [NCLS, 1], F32, kind="ExternalInput")
    exp_end_t = dt("exp_end", [NCLS, 1], F32, kind="ExternalInput")
    expT_t = dt("expT", [NCLS, NCLS], F32, kind="ExternalInput")
    ones_t = dt("ones25", [NCLS, NCLS], F32, kind="ExternalInput")
    U8 = mybir.dt.uint8
    mask_t = dt("mask25", [NCLS, max(nmask, 1) * BS], U8,
                kind="ExternalInput")
    emis_out = dt("emis16", [NCLS, ntok], F16, kind="ExternalOutput")
    crf_out = dt("crf", [1, BS], F32, kind="ExternalOutput")

    with tile.TileContext(nc) as tc:
        with ExitStack() as ctx:
            ep = ctx.enter_context
            cpool = ep(tc.tile_pool(name="const", bufs=1))
            dram = ep(tc.tile_pool(name="dram", bufs=1, space="DRAM"))

            def load(name, src, shape, dtyp):
                t_ = cpool.tile(shape, dtyp, tag=name)
                nc.sync.dma_start(out=t_[:], in_=src[:])
                return t_

            wih_sb = [load(f"wih{d}", wihT[d], [128, 2 * 4 * H], F16)
                      for d in range(2)]
            whh_sb = [load(f"whh{d}", whhT[d], [128, 2 * 4 * H], F16)
                      for d in range(2)]
            bias_sb = [load(f"bias{d}", bias[d], [128, 8], F32)
                       for d in range(2)]
            weT_sb = load("weT", weT, [128, 4 * NCLS], F16)
            b_e_sb = load("b_e", b_e_t, [NCLS, 1], F32)
            exp_start_sb = load("exp_start", exp_start_t, [NCLS, 1], F32)
            exp_end_sb = load("exp_end", exp_end_t, [NCLS, 1], F32)
            expT_sb = load("expT", expT_t, [NCLS, NCLS], F32)
            ones_sb = load("ones25", ones_t, [NCLS, NCLS], F32)
            mask_sb = load("mask25", mask_t, [NCLS, max(nmask, 1) * BS], U8)

            h_hist = [cpool.tile([128, t_dev * 2 * BS], F16, tag=f"hh{d}", name=f"hh{d}")
                      for d in range(2)]
            zeros_sb = cpool.tile([128, 2 * BS], F16, tag="zeros", name="zeros")
            nc.vector.memset(zeros_sb[:], 0.0)

            xg_dram = [dram.tile([nblk, 128, 8 * 512], F16, tag=f"xgd{d}", name=f"xgd{d}")
                       for d in range(2)]

            # ---- phase A: x^T via DMA transpose ----
            xT = [cpool.tile([128, ntok], F16, tag=f"xT{k}", name=f"xT{k}") for k in range(2)]
            for k in range(2):
                nc.sync.dma_start_transpose(out=xT[k][:], in_=x16[k])

            # ---- phase B: xg = W_ih @ x^T + b -> DRAM (f16) ----
            with ExitStack() as bctx:
                xp = bctx.enter_context(
                    tc.tile_pool(name="xgp", bufs=4, space="PSUM"))
                xs = bctx.enter_context(tc.tile_pool(name="xgs", bufs=2))
                for d in range(2):
                    for n in range(nblk):
                        st = xs.tile([128, 8 * 512], F16, tag="xst", name="xst")
                        for m in range(8):
                            ps = xp.tile([128, 512], F32, tag="xg", name="xg")
                            for k in range(2):
                                nc.tensor.matmul(
                                    out=ps[:],
                                    lhsT=wih_sb[d][:, k * 1024 + m * 128:
                                                   k * 1024 + (m + 1) * 128],
                                    rhs=xT[k][:, n * 512:(n + 1) * 512],
                                    start=(k == 0), stop=(k == 1))
                            dst = st[:, m * 512:(m + 1) * 512]
                            if m % 2 == 0:
                                nc.scalar.activation(
                                    out=dst, in_=ps[:], func=AF.Identity,
                                    bias=bias_sb[d][:, m:m + 1])
                            else:
                                nc.vector.tensor_scalar_add(
                                    dst, ps[:], bias_sb[d][:, m:m + 1])
                        nc.sync.dma_start(out=xg_dram[d][n], in_=st[:])

            # ---- phase C: BiLSTM scan ----
            with ExitStack() as cctx:
                pp = [cctx.enter_context(
                    tc.tile_pool(name=f"gp{d}", bufs=2, space="PSUM"))
                    for d in range(2)]
                wp = cctx.enter_context(tc.tile_pool(name="work", bufs=3))
                xb = cctx.enter_context(tc.tile_pool(name="xgb", bufs=2))
                cst = cctx.enter_context(tc.tile_pool(name="cstate", bufs=1))
                c_sb = [cst.tile([128, 2 * BS], F32, tag=f"c{d}", name=f"c{d}")
                        for d in range(2)]
                for d in range(2):
                    nc.vector.memset(c_sb[d][:], 0.0)

                xg_buf = [[None, None] for _ in range(2)]

                def step(d, t):
                    blk_i = t // BLK
                    sl = blk_i % 2
                    if (t % BLK == 0) if d == 0 else (t % BLK == BLK - 1):
                        bt = xb.tile([128, 8 * 512], F16, tag=f"xgb{d}", name=f"xgb{d}")
                        nc.sync.dma_start(out=bt[:], in_=xg_dram[d][blk_i])
                        xg_buf[d][sl] = bt
                    buf = xg_buf[d][sl]
                    c_loc = t % BLK
                    ps = pp[d].tile([128, 8 * BS], F32, tag=f"g{d}", name=f"g{d}")
                    if (t == 0 and d == 0) or (t == t_dev - 1 and d == 1):
                        h_prev, hoff = zeros_sb, 0
                    else:
                        h_prev = h_hist[d]
                        hoff = (t - 1 if d == 0 else t + 1) * 2 * BS
                    for m in range(8):
                        for k in range(2):
                            nc.tensor.matmul(
                                out=ps[:, m * BS:(m + 1) * BS],
                                lhsT=whh_sb[d][:, k * 1024 + m * 128:
                                               k * 1024 + (m + 1) * 128],
                                rhs=h_prev[:, hoff + k * BS:hoff + (k + 1) * BS],
                                start=(k == 0), stop=(k == 1))
                    xg_ap = buf[:].rearrange(
                        "p (m c b) -> p m c b", m=8, c=BLK)[
                        :, :, c_loc, :]
                    ps_v = ps[:].rearrange("p (m b) -> p m b", m=8)
                    nc.vector.tensor_add(out=ps_v, in0=ps_v, in1=xg_ap)
                    sig = wp.tile([128, 6 * BS], F32, tag=f"sig{d}", name=f"sig{d}")
                    nc.scalar.activation(out=sig[:], in_=ps[:, 0:6 * BS],
                                         func=AF.Sigmoid)
                    tg = wp.tile([128, 2 * BS], F32, tag=f"tg{d}", name=f"tg{d}")
                    nc.scalar.activation(out=tg[:], in_=ps[:, 6 * BS:8 * BS],
                                         func=AF.Tanh)
                    u = wp.tile([128, 2 * BS], F32, tag=f"u{d}", name=f"u{d}")
                    nc.gpsimd.tensor_mul(out=u[:], in0=sig[:, 0:2 * BS],
                                         in1=tg[:])
                    ct = wp.tile([128, 2 * BS], F32, tag=f"ct{d}", name=f"ct{d}")
                    nc.vector.tensor_mul(out=ct[:], in0=sig[:, 2 * BS:4 * BS],
                                         in1=c_sb[d][:])
                    nc.vector.tensor_add(out=c_sb[d][:], in0=ct[:], in1=u[:])
                    th = wp.tile([128, 2 * BS], F32, tag=f"th{d}", name=f"th{d}")
                    nc.scalar.activation(out=th[:], in_=c_sb[d][:],
                                         func=AF.Tanh)
                    nc.gpsimd.tensor_mul(
                        out=h_hist[d][:, t * 2 * BS:(t + 1) * 2 * BS],
                        in0=sig[:, 4 * BS:6 * BS], in1=th[:])

                for r in range(t_dev):
                    step(0, r)
                    step(1, t_dev - 1 - r)

            # ---- phase D: emissions + exp ----
            exp_emis = cpool.tile([NCLS, ntok], F32, tag="expem", name="expem")
            emis_sb = cpool.tile([NCLS, ntok], F16, tag="emis16", name="emis16")
            with ExitStack() as dctx:
                epp = dctx.enter_context(
                    tc.tile_pool(name="emp", bufs=4, space="PSUM"))
                for n in range(nblk):
                    ps = epp.tile([NCLS, 512], F32, tag="em", name="em")
                    for kk in range(4):
                        hist = h_hist[kk // 2]
                        rhs = hist[:].rearrange(
                            "p (t k b) -> p t k b", k=2, b=BS)[
                            :, n * BLK:(n + 1) * BLK, kk % 2, :]
                        nc.tensor.matmul(
                            out=ps[:],
                            lhsT=weT_sb[:, kk * NCLS:(kk + 1) * NCLS],
                            rhs=rhs, start=(kk == 0), stop=(kk == 3))
                    nc.scalar.activation(
                        out=exp_emis[:, n * 512:(n + 1) * 512], in_=ps[:],
                        func=AF.Exp, bias=b_e_sb[:])
                    nc.vector.tensor_scalar_add(
                        emis_sb[:, n * 512:(n + 1) * 512], ps[:], b_e_sb[:])
            nc.sync.dma_start(out=emis_out[:], in_=emis_sb[:])

            # ---- phase E: CRF forward scan in exp space ----
            Eb = cpool.tile([NCLS, BS], F32, tag="Eb", name="Eb")
            acc = cpool.tile([1, BS], F32, tag="acc", name="acc")
            nc.vector.memset(acc[:], 0.0)
            nc.scalar.activation(out=Eb[:], in_=exp_emis[:, 0:BS],
                                 func=AF.Copy, scale=exp_start_sb[:])
            with ExitStack() as ectx:
                cp = ectx.enter_context(
                    tc.tile_pool(name="crfp", bufs=2, space="PSUM"))
                cwp = ectx.enter_context(tc.tile_pool(name="crfw", bufs=2))

                def renorm_sum():
                    # (25, BS) column sums replicated on all partitions
                    pss = cp.tile([NCLS, BS], F32, tag="crfsum", name="crfsum")
                    nc.tensor.matmul(out=pss[:], lhsT=ones_sb[:], rhs=Eb[:],
                                     start=True, stop=True)
                    return pss

                for t in range(1, t_dev):
                    ps = cp.tile([NCLS, BS], F32, tag="crf", name="crf")
                    nc.tensor.matmul(out=ps[:], lhsT=expT_sb[:], rhs=Eb[:],
                                     start=True, stop=True)
                    em = exp_emis[:, t * BS:(t + 1) * BS]
                    if t < tmin:
                        nc.vector.tensor_mul(out=Eb[:], in0=ps[:], in1=em)
                    else:
                        tm = cwp.tile([NCLS, BS], F32, tag="tmE", name="tmE")
                        nc.vector.tensor_mul(out=tm[:], in0=ps[:], in1=em)
                        moff = (t - tmin) * BS
                        nc.vector.copy_predicated(
                            out=Eb[:], mask=mask_sb[:, moff:moff + BS],
                            data=tm[:])
                    if t % RENORM == 0:
                        pss = renorm_sum()
                        rec = cwp.tile([NCLS, BS], F32, tag="rec", name="rec")
                        nc.vector.reciprocal(out=rec[:], in_=pss[:])
                        lg = cwp.tile([1, BS], F32, tag="lg", name="lg")
                        nc.scalar.activation(out=lg[:], in_=pss[0:1, :],
                                             func=AF.Ln)
                        nc.vector.tensor_add(out=acc[:], in0=acc[:], in1=lg[:])
                        nc.vector.tensor_mul(out=Eb[:], in0=Eb[:], in1=rec[:])

                # final: denom = acc + ln(sum_i E_i * exp_end_i)
                Ee = cwp.tile([NCLS, BS], F32, tag="Ee", name="Ee")
                nc.scalar.activation(out=Ee[:], in_=Eb[:], func=AF.Copy,
                                     scale=exp_end_sb[:])
                pss = cp.tile([NCLS, BS], F32, tag="crfsum", name="crfsum")
                nc.tensor.matmul(out=pss[:], lhsT=ones_sb[:], rhs=Ee[:],
                                 start=True, stop=True)
                lg = cwp.tile([1, BS], F32, tag="lg", name="lg")
                nc.scalar.activation(out=lg[:], in_=pss[0:1, :], func=AF.Ln)
                res = cwp.tile([1, BS], F32, tag="res", name="res")
                nc.vector.tensor_add(out=res[:], in0=acc[:], in1=lg[:])
                nc.sync.dma_start(out=crf_out[:], in_=res[:])
    nc.finalize()
    return nc


def _prep_host(sentence, emb, w_ih_f, w_hh_f, b_ih_f, b_hh_f,
               w_ih_b, w_hh_b, b_ih_b, b_hh_b,
               W_e, b_e, start_trans, end_trans, trans,
               t_dev=T, tmin=TMIN, ncores=NCORES):
    F16 = np.float16
    perm = _gate_perm()
    ntok = t_dev * BS

    def pack_w(w):  # (4H, K) -> (128, 2*4H) f16 lhsT tiles
        a = np.ascontiguousarray(w[perm].T.astype(np.float32))  # (K, 4H)
        return np.concatenate([a[0:128], a[128:256]], axis=1).astype(F16)

    def pack_bias(bi, bh):
        bb = (np.asarray(bi) + np.asarray(bh)).astype(np.float32)[perm]
        return np.ascontiguousarray(bb.reshape(8, 128).T)  # (128, 8)

    weTf = np.asarray(W_e).astype(np.float32).T  # (512, 25)
    weT = np.concatenate([weTf[k * 128:(k + 1) * 128] for k in range(4)],
                         axis=1).astype(F16)  # (128, 100)
    common = dict(
        wihT0=pack_w(np.asarray(w_ih_f)), wihT1=pack_w(np.asarray(w_ih_b)),
        whhT0=pack_w(np.asarray(w_hh_f)), whhT1=pack_w(np.asarray(w_hh_b)),
        bias0=pack_bias(b_ih_f, b_hh_f), bias1=pack_bias(b_ih_b, b_hh_b),
        weT=weT,
        b_e=np.asarray(b_e).astype(np.float32).reshape(NCLS, 1),
        exp_start=np.exp(np.asarray(start_trans).astype(np.float32))
        .reshape(NCLS, 1),
        exp_end=np.exp(np.asarray(end_trans).astype(np.float32))
        .reshape(NCLS, 1),
        expT=np.ascontiguousarray(
            np.exp(np.asarray(trans).astype(np.float32))),
        ones25=np.ones((NCLS, NCLS), np.float32),
    )
    emb16 = np.asarray(emb).astype(F16)
    mask = (sentence != PAD)
    nmask = max(t_dev - tmin, 1)
    in_maps = []
    for kc in range(ncores):
        sh = sentence[:, kc * BS:(kc + 1) * BS]  # (t_dev, BS)
        xg = emb16[sh.reshape(-1)]               # (ntok, EMB) f16
        x = np.stack([xg[:, :128], xg[:, 128:]])  # (2, ntok, 128)
        mT = mask[tmin:, kc * BS:(kc + 1) * BS].astype(np.uint8)
        m25 = np.broadcast_to(mT.reshape(1, -1), (NCLS, mT.size))
        if mT.size == 0:
            m25 = np.zeros((NCLS, nmask * BS), np.uint8)
        in_maps.append(dict(common, x16=np.ascontiguousarray(x),
                            mask25=np.ascontiguousarray(m25)))
    return in_maps, mask


def kernel(sentence, tags, emb,
           w_ih_f, w_hh_f, b_ih_f, b_hh_f,
           w_ih_b, w_hh_b, b_ih_b, b_hh_b,
           W_e, b_e, start_trans, end_trans, trans):
    from concourse.bass_utils import run_bass_kernel_spmd
    sentence = np.asarray(sentence)
    tags = np.asarray(tags).astype(np.int64)
    in_maps, mask = _prep_host(
        sentence, np.asarray(emb), w_ih_f, w_hh_f, b_ih_f, b_hh_f,
        w_ih_b, w_hh_b, b_ih_b, b_hh_b, W_e, b_e,
        start_trans, end_trans, trans)
    if "nc" not in _CACHE:
        _CACHE["nc"] = _build_bass()
    res = run_bass_kernel_spmd(_CACHE["nc"], in_maps, list(range(NCORES)))

    emis = np.empty((T, B, NCLS), np.float32)
    denom = np.empty((B,), np.float64)
    for kc in range(NCORES):
        e16 = np.asarray(res.results[kc]["emis16"])    # (25, NTOK)
        emis[:, kc * BS:(kc + 1) * BS, :] = (
            e16.astype(np.float32).reshape(NCLS, T, BS).transpose(1, 2, 0))
        denom[kc * BS:(kc + 1) * BS] = np.asarray(
            res.results[kc]["crf"]).reshape(BS)

    f32 = np.float32
    st_, et_, tr_ = (np.asarray(start_trans, f32), np.asarray(end_trans, f32),
                     np.asarray(trans, f32))
    mf = mask.astype(f32)
    bar = np.arange(B)
    emis_at = np.take_along_axis(emis, tags[..., None], axis=-1)[..., 0]
    num = st_[tags[0]] + emis_at[0]
    trans_sc = tr_[tags[:-1], tags[1:]]
    num = num + np.sum(mf[1:] * (trans_sc + emis_at[1:]), axis=0)
    seq_ends = np.sum(mask, axis=0) - 1
    last_tags = tags[seq_ends, bar]
    num = num + et_[last_tags]

    llh = num.astype(np.float64) - denom
    return np.float32(-np.sum(llh))
